# revision 7
# baseline (speedup 1.0000x reference)
"""DeepSeekMoE forward on 8 TRN2 NeuronCores.

Strategy (expert-parallel, per the sharding hint):
  - Host computes the (tiny) gate: scores = sqrt(softplus(x @ gate_w)),
    top-2 selection, normalized combine weights, and builds per-expert
    token lists (the "all-to-all dispatch" done host-side since kernel()
    receives full inputs and returns the full output).
  - Core e holds routed expert e's weights and processes the tokens
    routed to it (padded to a common capacity C).
  - The shared expert is split along its intermediate dim I across the
    8 cores (each core computes a 384-wide slice for ALL tokens); the
    partial outputs sum to the exact shared-expert output.
  - Host scatters/sums the per-core outputs back to [B, T, D].

Device compute is bf16 (f32 PSUM accumulation): TRN2 PE does bf16 at
1 cycle/row vs 4 for fp32, and bf16 halves the HBM traffic.

Perf notes (from perfetto/NTFF iteration; baseline of this round was
~166-170 us with PE 88.6% busy and streaming roofline ~150 us):
  - each sync.dma_start costs ~600 ns of Sync-engine issue time, so
    DMAs are batched: one dma_start per w1/w3 slab PAIR (host packs the
    pair contiguously per partition), xg in two chunk-halves, residents
    (xt/w2/w2s) in 2-3 big pieces on the SCALAR HWDGE queue (separate
    hardware queue -> no head-of-line blocking of the slab FIFO);
  - C pads token counts to 8 (not 32) - matmul N can be any length;
  - outputs are staged in SBUF as bf16 and written with one dma_start
    per token chunk ([128,768] for the shared partial, [128,6,tl] for
    the routed chunk), cutting the post-matmul drain tail;
  - dummy matmuls on a zeroed tile (gpsimd memset, earliest engine
    available) warm the PE HAM clock gate (1.2 -> 2.4 GHz) while the
    first DMAs land;
  - evict-heavy shared GEMM2 units interleave with evict-light routed
    GEMM2 units so the DVE/ACT eviction pipeline drains under PE;
  - token chunks are split EQUALLY (280/280 not 512/48): a sliver
    chunk's matmuls are LDWEIGHTS-bound (53 ns load vs 20 ns matmul).
"""

import math

import numpy as np
import ml_dtypes

import concourse.bass as bass
import concourse.tile as tile
from concourse import bacc, mybir
from concourse.bass_utils import run_bass_kernel_spmd

BF16 = np.dtype(ml_dtypes.bfloat16)
DT_BF16 = mybir.dt.bfloat16
DT_F32 = mybir.dt.float32

D = 768            # n_embd
I = 3072           # moe_intermediate_size
E = 8              # n_routed_experts
TOPK = 2
LIMIT = 10.0
NTOK = 2048        # B*T
NCORES = 8
ISH = I // NCORES  # shared-expert I slice per core (384)
DTILES = D // 128  # 6
MI = I // 128      # 24 routed i-tiles
MS = ISH // 128    # 3 shared i-tiles
NTC = NTOK // 128  # 16 shared token chunks

_BUILD_CACHE: dict = {}
last_results = None  # BassKernelResults of the most recent run (for test.py)
DEBUG_DUMP = False   # add xg_sb/h_sb dump outputs to the graph


def _chunks(total, step=512):
    # Balanced chunking: a trailing sliver (e.g. 48 wide) makes its
    # matmuls LDWEIGHTS-bound; equal chunks keep every matmul long
    # enough to hide the stationary loads.
    n = max(1, math.ceil(total / step))
    base = total // n
    rem = total - base * n
    out, t0 = [], 0
    for i in range(n):
        ln = base + (1 if i < rem else 0)
        out.append((t0, ln))
        t0 += ln
    return out


def _build(C):
    """Build the SPMD Bass graph for capacity C (tokens per routed expert)."""
    nc = bacc.Bacc("TRN2", target_bir_lowering=False, debug=False)

    ap = lambda name, shape, dt, kind: nc.dram_tensor(name, shape, dt, kind=kind).ap()
    w13 = ap("w13", [MI, 128, 2, DTILES, 128], DT_BF16, "ExternalInput")
    w2 = ap("w2", [128, MI, D], DT_BF16, "ExternalInput")
    w13s = ap("w13s", [MS, 128, 2, DTILES, 128], DT_BF16, "ExternalInput")
    w2s = ap("w2s", [128, MS, D], DT_BF16, "ExternalInput")
    xt = ap("xt", [128, DTILES, NTOK], DT_BF16, "ExternalInput")
    xg = ap("xg", [128, DTILES, C], DT_BF16, "ExternalInput")
    # Outputs are partition-major so eviction staging tiles DMA out with
    # maximal per-partition contiguity; the host untiles them.
    out_r = ap("out_r", [128, DTILES, C], DT_BF16, "ExternalOutput")
    out_s = ap("out_s", [128, NTC, D], DT_BF16, "ExternalOutput")
    if DEBUG_DUMP:
        xg_dbg = ap("xg_dbg", [128, DTILES, C], DT_BF16, "ExternalOutput")
        h_dbg = ap("h_dbg", [128, MI, C], DT_BF16, "ExternalOutput")

    TCR = _chunks(C)      # routed token chunks (280/280 for C=560)
    TCS = _chunks(NTOK)   # shared token chunks (512 x 4)
    DC = _chunks(D)       # output d chunks (512, 256)

    MIN = mybir.AluOpType.min
    MAX = mybir.AluOpType.max
    SILU = mybir.ActivationFunctionType.Silu
    COPY = mybir.ActivationFunctionType.Copy

    with tile.TileContext(nc) as tc:
        with (
            tc.tile_pool(name="res", bufs=1) as res,
            tc.tile_pool(name="slab", bufs=4) as slabs,
            tc.tile_pool(name="tmp", bufs=4) as tmps,
            tc.tile_pool(name="stgr", bufs=2) as stgr,
            tc.tile_pool(name="stgs", bufs=3) as stgs,
            tc.tile_pool(name="ps", bufs=8, space="PSUM") as ps1,
        ):
            # PE warm-up: the HAM clock gate needs ~3.4us of sustained
            # activity to lift the PE from 1.2 to 2.4 GHz. memset on
            # gpsimd (first engine free after the framework preamble),
            # then dummy matmuls while the first DMAs land.
            warm = res.tile([128, 512], DT_BF16)
            nc.gpsimd.memset(warm[:], 0.0)
            pw = ps1.tile([128, 512], DT_F32, tag="ps", name="pw")
            for i in range(8):
                nc.tensor.matmul(pw[:], warm[:, :128], warm[:],
                                 start=(i == 0), stop=(i == 7))

            xg_sb = res.tile([128, DTILES, C], DT_BF16)
            xt_sb = res.tile([128, DTILES, NTOK], DT_BF16)
            w2_sb = res.tile([128, MI, D], DT_BF16)
            w2s_sb = res.tile([128, MS, D], DT_BF16)
            h_sb = res.tile([128, MI, C], DT_BF16)
            hs_sb = res.tile([128, MS, NTOK], DT_BF16)

            def gemm1(npairs, wsrc, x_sb, tchunks, hout, side_loads={}):
                # hout[i, t] = silu(min(W1.T x, L)) * clip(W3.T x, -L, L)
                # side_loads: {(m, -1): fns before iteration m's slab DMA,
                #              (m, ci): fns before chunk ci of iteration m}.
                # Loads for data read inside this loop MUST be keyed so they
                # precede the first reader in program order.
                for m in range(npairs):
                    for fn in side_loads.get((m, -1), []):
                        fn()
                    pair = slabs.tile([128, 2, DTILES, 128], DT_BF16, tag="slab")
                    nc.sync.dma_start(pair[:], wsrc[m])
                    sg = pair[:, 0]
                    su = pair[:, 1]
                    for ci, (t0, tl) in enumerate(tchunks):
                        for fn in side_loads.get((m, ci), []):
                            fn()
                        pg = ps1.tile([128, 512], DT_F32, tag="ps", name="pg")[:, :tl]
                        pu = ps1.tile([128, 512], DT_F32, tag="ps", name="pu")[:, :tl]
                        for d in range(DTILES):
                            nc.tensor.matmul(
                                pg[:], sg[:, d, :], x_sb[:, d, t0:t0 + tl],
                                start=(d == 0), stop=(d == DTILES - 1))
                        for d in range(DTILES):
                            nc.tensor.matmul(
                                pu[:], su[:, d, :], x_sb[:, d, t0:t0 + tl],
                                start=(d == 0), stop=(d == DTILES - 1))
                        tg = tmps.tile([128, 512], DT_F32, tag="tg", name="tg")[:, :tl]
                        nc.vector.tensor_scalar(tg[:], pg[:], LIMIT, None, MIN)
                        sa = tmps.tile([128, 512], DT_F32, tag="sa", name="sa")[:, :tl]
                        nc.scalar.activation(sa[:], tg[:], SILU)
                        tu = tmps.tile([128, 512], DT_F32, tag="tu", name="tu")[:, :tl]
                        nc.vector.tensor_scalar(tu[:], pu[:], LIMIT, -LIMIT, MIN, MAX)
                        nc.vector.tensor_mul(hout[:, m, t0:t0 + tl], sa[:], tu[:])

            ev_flip = [0]

            def evict(dst, src):
                # Alternate eviction engines so DVE and ACT split the
                # PSUM->SBUF drain work.
                if ev_flip[0] % 2 == 0:
                    nc.vector.tensor_copy(dst, src)
                else:
                    nc.scalar.activation(dst, src, COPY)
                ev_flip[0] += 1

            def gemm2T_units(nitiles, h, w2sb, tlen_total, dst):
                # dst[:, dt, t] = (w2.T @ h) tile - transposed output
                # layout; PE cost scales with tlen_total itself, not its
                # 128-padded tiles. Combine-weight scaling happens host-side.
                for (t0, tl) in _chunks(tlen_total):
                    stage = stgr.tile([128, DTILES, tl], DT_BF16, tag="str",
                                      name=f"str{t0}")
                    for dt_ in range(DTILES):
                        def unit(t0=t0, tl=tl, dt_=dt_, stage=stage):
                            ps = ps1.tile([128, 512], DT_F32, tag="ps", name="pt")[:, :tl]
                            for m in range(nitiles):
                                nc.tensor.matmul(
                                    ps[:], w2sb[:, m, dt_ * 128:(dt_ + 1) * 128],
                                    h[:, m, t0:t0 + tl],
                                    start=(m == 0), stop=(m == nitiles - 1))
                            evict(stage[:, dt_, :], ps[:])
                            if dt_ == DTILES - 1:
                                nc.sync.dma_start(dst[:, :, t0:t0 + tl], stage[:])
                        yield unit

            def gemm2_units(nitiles, h, w2sb, dst):
                # dst[:, tt, d] = (h.T @ w2) token-chunk tile
                for tt in range(NTC):
                    t0 = tt * 128
                    stage = stgs.tile([128, D], DT_BF16, tag="sts", name=f"sts{tt}")
                    for di, (d0, dl) in enumerate(DC):
                        def unit(tt=tt, t0=t0, di=di, d0=d0, dl=dl, stage=stage):
                            ps = ps1.tile([128, 512], DT_F32, tag="ps", name="po")[:, :dl]
                            for m in range(nitiles):
                                nc.tensor.matmul(
                                    ps[:], h[:, m, t0:t0 + 128],
                                    w2sb[:, m, d0:d0 + dl],
                                    start=(m == 0), stop=(m == nitiles - 1))
                            evict(stage[:, d0:d0 + dl], ps[:])
                            if di == len(DC) - 1:
                                nc.sync.dma_start(dst[:, tt, :], stage[:])
                        yield unit

            # xg chunk 0 lands before slab pair 0 in the sync DMA FIFO and
            # xg chunk 1 is issued between chunk 0's and chunk 1's matmuls
            # of iteration 0 (program order must put the write before the
            # first reader!), so the first real matmul's prerequisites
            # (slab pair 0 + xg chunk 0) land as early as possible.
            # Residents stream on the scalar HWDGE queue (separate HW
            # queue; Scalar engine has slack between silu activations).
            def xg_load(ci):
                t0, tl = TCR[ci]
                return lambda: nc.sync.dma_start(
                    xg_sb[:, :, t0:t0 + tl], xg[:, :, t0:t0 + tl])
            side = {
                (0, -1): [xg_load(0)],
                (3, -1): [lambda: nc.scalar.dma_start(
                    xt_sb[:, 0:3, :], xt[:, 0:3, :])],
                (7, -1): [lambda: nc.scalar.dma_start(
                    xt_sb[:, 3:6, :], xt[:, 3:6, :])],
                (11, -1): [lambda: nc.scalar.dma_start(
                    w2_sb[:, 0:12, :], w2[:, 0:12, :])],
                (15, -1): [lambda: nc.scalar.dma_start(
                    w2_sb[:, 12:24, :], w2[:, 12:24, :])],
                (19, -1): [lambda: nc.scalar.dma_start(w2s_sb[:], w2s[:])],
            }
            for ci in range(1, len(TCR)):
                side[(0, ci)] = [xg_load(ci)]
            gemm1(MI, w13, xg_sb, TCR, h_sb, side)
            gemm1(MS, w13s, xt_sb, TCS, hs_sb)
            if DEBUG_DUMP:
                nc.sync.dma_start(xg_dbg[:], xg_sb[:])
                nc.sync.dma_start(h_dbg[:], h_sb[:])
            # Interleave the evict-heavy shared GEMM2 (many small psum
            # groups) with the evict-light routed GEMM2 (long psum
            # accumulations) so the eviction pipeline drains while PE is
            # still busy, and the kernel ends on an evict-light unit.
            r_units = list(gemm2T_units(MI, h_sb, w2_sb, C, out_r))
            s_units = list(gemm2_units(MS, hs_sb, w2s_sb, out_s))
            ns, nr = len(s_units), len(r_units)
            si = 0
            for ri, ru in enumerate(r_units):
                take = (ns * (ri + 1)) // nr
                while si < min(take, ns):
                    s_units[si]()
                    si += 1
                ru()
            while si < ns:
                s_units[si]()
                si += 1

    nc.compile()
    return nc


def _slabify(w):
    """[768, ncols] -> [ncols//128, 128, 6, 128] stationary slabs.

    slab[m, p, a, f] = w[a*128 + p, m*128 + f]
    """
    ncols = w.shape[1]
    return np.ascontiguousarray(
        w.reshape(DTILES, 128, ncols // 128, 128).transpose(2, 1, 0, 3))


def _pair_slabs(wg, wu):
    """Two [768, ncols] mats -> [ncols//128, 128, 2, 6, 128] slab pairs.

    Each pair is contiguous per partition (3072 B) so one dma_start
    streams a full w1/w3 slab pair.
    """
    m = wg.shape[1] // 128
    out = np.empty((m, 128, 2, DTILES, 128), dtype=BF16)
    out[:, :, 0] = _slabify(wg)
    out[:, :, 1] = _slabify(wu)
    return out


def _ptile(a):
    """[R, cols] with R = n*128 -> [128, n, cols] (partition-major)."""
    r, c = a.shape
    return np.ascontiguousarray(a.reshape(r // 128, 128, c).transpose(1, 0, 2))


def kernel(**inputs) -> np.ndarray:
    global last_results
    x = np.asarray(inputs["x"], dtype=np.float32)
    gate_w = np.asarray(inputs["gate_w"], dtype=np.float32)
    gate_bias = np.asarray(inputs["gate_bias"], dtype=np.float32)
    w1 = np.asarray(inputs["w1"], dtype=np.float32)
    w2 = np.asarray(inputs["w2"], dtype=np.float32)
    w3 = np.asarray(inputs["w3"], dtype=np.float32)
    w1s = np.asarray(inputs["w1s"], dtype=np.float32)
    w2s = np.asarray(inputs["w2s"], dtype=np.float32)
    w3s = np.asarray(inputs["w3s"], dtype=np.float32)

    B, T, _ = x.shape
    N = B * T
    assert N == NTOK, f"kernel compiled for {NTOK} tokens, got {N}"
    flat = x.reshape(N, D)

    # ---- gate (host, f32, mirrors reference semantics) ----
    logits = flat @ gate_w                              # [N, E]
    scores = np.sqrt(np.logaddexp(np.float32(0.0), logits)).astype(np.float32)
    routed = scores + gate_bias
    idx = np.argsort(-routed, axis=1, kind="stable")[:, :TOPK]      # [N, K]
    wts = np.take_along_axis(scores, idx, axis=1)
    wts = wts / np.clip(wts.sum(axis=1, keepdims=True), 1e-6, None)

    # ---- dispatch: per-expert token lists ----
    ee = idx.reshape(-1)
    tok = np.repeat(np.arange(N), TOPK)
    ww = wts.reshape(-1).astype(np.float32)
    toks, cwts, counts = [], [], []
    for e in range(E):
        sel = ee == e
        toks.append(tok[sel])
        cwts.append(ww[sel])
        counts.append(int(sel.sum()))
    C = max(128, ((max(counts) + 7) // 8) * 8)

    # ---- per-core input maps ----
    xt_h = _ptile(flat.T.astype(BF16))                  # [128, 6, N]
    in_maps = []
    for e in range(E):
        ce = counts[e]
        xg_full = np.zeros((C, D), dtype=np.float32)
        xg_full[:ce] = flat[toks[e]]
        sl = slice(e * ISH, (e + 1) * ISH)
        in_maps.append({
            "w13": _pair_slabs(w1[e].astype(BF16), w3[e].astype(BF16)),
            "w2": _ptile(w2[e].astype(BF16)),           # [128, 24, 768]
            "w13s": _pair_slabs(w1s[:, sl].astype(BF16), w3s[:, sl].astype(BF16)),
            "w2s": _ptile(w2s[sl].astype(BF16)),        # [128, 3, 768]
            "xt": xt_h,
            "xg": _ptile(xg_full.T.astype(BF16)),       # [128, 6, C]
        })

    # ---- build + run ----
    if C not in _BUILD_CACHE:
        _BUILD_CACHE[C] = _build(C)
    nc = _BUILD_CACHE[C]
    last_results = run_bass_kernel_spmd(nc, in_maps, core_ids=list(range(NCORES)))
    res = last_results.results

    # ---- combine (host): sum shared partials, scatter routed outputs ----
    # out_s: [128, 16, 768] partition-major bf16 partials -> [2048, 768] f32
    acc = res[0]["out_s"].astype(np.float32)
    for c in range(1, NCORES):
        acc += res[c]["out_s"].astype(np.float32)
    out = np.ascontiguousarray(acc.transpose(1, 0, 2)).reshape(N, D)
    for e in range(E):
        ce = counts[e]
        if ce:
            # out_r: [128, 6, C] bf16 -> [768, C] f32
            orr = res[e]["out_r"].astype(np.float32).transpose(1, 0, 2).reshape(D, C)
            out[toks[e]] += orr[:, :ce].T * cwts[e][:, None]
    return out.reshape(B, T, D).astype(np.float32)


# revision 10
# speedup vs baseline: 1.0913x; 1.0913x over previous
"""DeepSeekMoE forward on 8 TRN2 NeuronCores.

Strategy (expert-parallel, per the sharding hint):
  - Host computes the (tiny) gate: scores = sqrt(softplus(x @ gate_w)),
    top-2 selection, normalized combine weights, and builds per-expert
    token lists (the "all-to-all dispatch" done host-side since kernel()
    receives full inputs and returns the full output).
  - Core e holds routed expert e's weights and processes the tokens
    routed to it (padded to a common capacity C).
  - The shared expert is split along its intermediate dim I across the
    8 cores (each core computes a 384-wide slice for ALL tokens); the
    partial outputs sum to the exact shared-expert output.
  - Host scatters/sums the per-core outputs back to [B, T, D].

Device compute is bf16 (f32 PSUM accumulation): TRN2 PE does bf16 at
1 cycle/row vs 4 for fp32, and bf16 halves the HBM traffic.

Perf notes (from perfetto/NTFF iteration):
  - ALL data DMAs ride the single Sync HWDGE queue: each dma_start fans
    out over the 16 HW DMA engines, which round-robin BETWEEN active
    queues packet-by-packet - a second queue with bigger packets
    starves the latency-critical slab stream (measured 453 ns/packet
    vs 25 ns back-to-back);
  - each sync.dma_start costs ~600 ns of Sync-engine issue time and the
    HW ring holds only ~4 in-flight DMAs, so transfers are batched:
    one dma_start per w1/w3 slab PAIR (host packs the pair contiguously
    per partition), x chunk-major ([nchunk,128,6,L] so each chunk is
    one contiguous per-partition run), residents in ~0.8 MB pieces
    injected between slab pairs (bounded head-of-line blocking);
  - stationary slabs are loaded ONCE per (m,d) and both token chunks
    stream through (d-outer loops) - halves LDWEIGHTS;
  - outputs evict as bf16 (halves write traffic; adds ~1e-4 rel err)
    with per-chunk DMAs sized so the kernel-end drain is one ~70 KB
    transfer;
  - dummy matmuls on a zeroed tile warm the PE HAM clock gate
    (1.2 -> 2.4 GHz) while the first DMAs land;
  - evict-heavy shared GEMM2 units interleave with evict-light routed
    GEMM2 units so the DVE/ACT eviction pipeline drains under PE.
"""

import math

import numpy as np
import ml_dtypes

import concourse.bass as bass
import concourse.tile as tile
from concourse import bacc, mybir
from concourse.bass_utils import run_bass_kernel_spmd

BF16 = np.dtype(ml_dtypes.bfloat16)
DT_BF16 = mybir.dt.bfloat16
DT_F32 = mybir.dt.float32

D = 768            # n_embd
I = 3072           # moe_intermediate_size
E = 8              # n_routed_experts
TOPK = 2
LIMIT = 10.0
NTOK = 2048        # B*T
NCORES = 8
ISH = I // NCORES  # shared-expert I slice per core (384)
DTILES = D // 128  # 6
MI = I // 128      # 24 routed i-tiles
MS = ISH // 128    # 3 shared i-tiles
NTC = NTOK // 128  # 16 shared token chunks
LS = 512           # shared gemm1 token chunk len
NSC = NTOK // LS   # 4 shared gemm1 chunks

_BUILD_CACHE: dict = {}
last_results = None  # BassKernelResults of the most recent run (for test.py)
DEBUG_DUMP = False   # add xg_sb/h_sb dump outputs to the graph


def _cap(maxcount):
    """Routed capacity: equal chunks of <=512, each a multiple of 8."""
    n = max(1, math.ceil(maxcount / 512))
    L = (((maxcount + n - 1) // n) + 7) // 8 * 8
    if n == 1:
        L = max(L, 128)
    return n, L


def _build(NC_, LC):
    """Build the SPMD Bass graph; routed capacity C = NC_ chunks of LC."""
    C = NC_ * LC
    nc = bacc.Bacc("TRN2", target_bir_lowering=False, debug=False)

    ap = lambda name, shape, dt, kind: nc.dram_tensor(name, shape, dt, kind=kind).ap()
    w13 = ap("w13", [MI, 128, 2, DTILES, 128], DT_BF16, "ExternalInput")
    w2 = ap("w2", [128, MI, D], DT_BF16, "ExternalInput")
    w13s = ap("w13s", [MS, 128, 2, DTILES, 128], DT_BF16, "ExternalInput")
    w2s = ap("w2s", [128, MS, D], DT_BF16, "ExternalInput")
    xt = ap("xt", [NSC, 128, DTILES, LS], DT_BF16, "ExternalInput")
    xg = ap("xg", [NC_, 128, DTILES, LC], DT_BF16, "ExternalInput")
    # Outputs are partition-major so eviction tiles DMA out with maximal
    # per-partition contiguity; the host untiles them.
    out_r = ap("out_r", [128, DTILES, C], DT_BF16, "ExternalOutput")
    out_s = ap("out_s", [128, NTC, D], DT_BF16, "ExternalOutput")
    if DEBUG_DUMP:
        h_dbg = ap("h_dbg", [128, MI, C], DT_BF16, "ExternalOutput")

    DC = _chunks_of(D, 512)   # output d chunks (512, 256)

    MIN = mybir.AluOpType.min
    MAX = mybir.AluOpType.max
    SILU = mybir.ActivationFunctionType.Silu
    COPY = mybir.ActivationFunctionType.Copy

    with tile.TileContext(nc) as tc:
        with (
            tc.tile_pool(name="res", bufs=1) as res,
            tc.tile_pool(name="slab", bufs=5) as slabs,
            tc.tile_pool(name="tmp", bufs=6) as tmps,
            tc.tile_pool(name="evr", bufs=3) as evr,
            tc.tile_pool(name="stgs", bufs=3) as stgs,
            tc.tile_pool(name="ps", bufs=8, space="PSUM") as ps1,
        ):
            # PE warm-up: the HAM clock gate needs ~3.4us of sustained
            # activity to lift the PE from 1.2 to 2.4 GHz. Dummy matmuls
            # cover the first DMAs' landing time.
            warm = res.tile([128, 512], DT_BF16)
            nc.gpsimd.memset(warm[:], 0.0)
            pw = ps1.tile([128, 512], DT_F32, tag="ps", name="pw")
            for i in range(7):
                nc.tensor.matmul(pw[:], warm[:, :128], warm[:],
                                 start=(i == 0), stop=(i == 6))

            xg_sb = [res.tile([128, DTILES, LC], DT_BF16, name=f"xgc{i}")
                     for i in range(NC_)]
            xt_sb = [res.tile([128, DTILES, LS], DT_BF16, name=f"xtc{i}")
                     for i in range(NSC)]
            w2_sb = res.tile([128, MI, D], DT_BF16)
            w2s_sb = res.tile([128, MS, D], DT_BF16)
            h_sb = res.tile([128, MI, C], DT_BF16)
            hs_sb = res.tile([128, MS, NTOK], DT_BF16)

            def gemm1(npairs, wsrc, xcs, clen, hout, side_loads={},
                      chunk_outer_m0=False):
                # hout[i, t] = silu(min(W1.T x, L)) * clip(W3.T x, -L, L)
                # d-outer: each stationary slab is LDWEIGHTS'd once and
                # both token chunks of the group stream through it.
                # side_loads: {(m, -1): before iter m's slab DMA,
                #              (m, ci): before chunk-group ci of iter m}
                # (writes must precede their first reader in program order).
                nch = len(xcs)
                for m in range(npairs):
                    for fn in side_loads.get((m, -1), []):
                        fn()
                    pair = slabs.tile([128, 2, DTILES, 128], DT_BF16, tag="slab")
                    nc.sync.dma_start(pair[:], wsrc[m])
                    groups = ([ [ci] for ci in range(nch) ]
                              if (chunk_outer_m0 and m == 0)
                              else [ list(range(g, min(g + 2, nch)))
                                     for g in range(0, nch, 2) ])
                    pgs = {}
                    for grp in groups:
                        for fn in side_loads.get((m, grp[0]), []):
                            fn()
                        for gi in range(2):
                            su = pair[:, gi]
                            pp = [ps1.tile([128, 512], DT_F32, tag="ps",
                                           name=f"pg{gi}")[:, :clen]
                                  for _ in grp]
                            for d in range(DTILES):
                                for p, ci in zip(pp, grp):
                                    nc.tensor.matmul(
                                        p[:], su[:, d, :], xcs[ci][:, d, :],
                                        start=(d == 0), stop=(d == DTILES - 1))
                            for p, ci in zip(pp, grp):
                                pgs[(gi, ci)] = p
                        for ci in grp:
                            pg, pu = pgs[(0, ci)], pgs[(1, ci)]
                            t0 = ci * clen
                            tg = tmps.tile([128, 512], DT_F32, tag="tg",
                                           name="tg")[:, :clen]
                            nc.vector.tensor_scalar(tg[:], pg[:], LIMIT, None, MIN)
                            sa = tmps.tile([128, 512], DT_F32, tag="sa",
                                           name="sa")[:, :clen]
                            nc.scalar.activation(sa[:], tg[:], SILU)
                            tu = tmps.tile([128, 512], DT_F32, tag="tu",
                                           name="tu")[:, :clen]
                            nc.vector.tensor_scalar(tu[:], pu[:], LIMIT, -LIMIT,
                                                    MIN, MAX)
                            nc.vector.tensor_mul(hout[:, m, t0:t0 + clen],
                                                 sa[:], tu[:])

            ev_flip = [0]

            def evict(dst, src):
                # Alternate eviction engines so DVE and ACT split the
                # PSUM->SBUF drain work.
                if ev_flip[0] % 2 == 0:
                    nc.vector.tensor_copy(dst, src)
                else:
                    nc.scalar.activation(dst, src, COPY)
                ev_flip[0] += 1

            def gemm2T_units():
                # out_r[:, dt, t] = (w2.T @ h) - transposed output layout;
                # each stationary w2 tile loads once, all routed token
                # chunks stream through (interleaved psum groups).
                for dt_ in range(DTILES):
                    def unit(dt_=dt_):
                        pp = [ps1.tile([128, 512], DT_F32, tag="ps",
                                       name="pt")[:, :LC] for _ in range(NC_)]
                        for m in range(MI):
                            st = w2_sb[:, m, dt_ * 128:(dt_ + 1) * 128]
                            for ci, p in enumerate(pp):
                                nc.tensor.matmul(
                                    p[:], st, h_sb[:, m, ci * LC:(ci + 1) * LC],
                                    start=(m == 0), stop=(m == MI - 1))
                        for ci, p in enumerate(pp):
                            ev = evr.tile([128, LC], DT_BF16, tag="evr",
                                          name="evr")
                            evict(ev[:], p[:])
                            nc.sync.dma_start(
                                out_r[:, dt_, ci * LC:(ci + 1) * LC], ev[:])
                    yield unit

            def gemm2_units():
                # out_s[:, tt, d] = (hs.T @ w2s) token-chunk tile; each
                # stationary hs tile loads once, both d chunks stream.
                for tt in range(NTC):
                    def unit(tt=tt):
                        stage = stgs.tile([128, D], DT_BF16, tag="sts",
                                          name=f"sts{tt}")
                        pp = [ps1.tile([128, 512], DT_F32, tag="ps",
                                       name="po")[:, :dl] for (d0, dl) in DC]
                        for m in range(MS):
                            st = hs_sb[:, m, tt * 128:(tt + 1) * 128]
                            for p, (d0, dl) in zip(pp, DC):
                                nc.tensor.matmul(
                                    p[:], st, w2s_sb[:, m, d0:d0 + dl],
                                    start=(m == 0), stop=(m == MS - 1))
                        for p, (d0, dl) in zip(pp, DC):
                            evict(stage[:, d0:d0 + dl], p[:])
                        nc.sync.dma_start(out_s[:, tt, :], stage[:])
                    yield unit

            # DMA FIFO plan (single sync queue; issue order = transfer
            # order): xg c0, pair0, xg c1.., then pairs with residents
            # (~0.8 MB pieces) injected between them.
            side = {(0, -1): [lambda: nc.sync.dma_start(xg_sb[0][:], xg[0])]}
            for ci in range(1, NC_):
                side[(0, ci)] = [lambda ci=ci: nc.sync.dma_start(
                    xg_sb[ci][:], xg[ci])]
            for i in range(NSC):
                side[(4 + 2 * i, -1)] = [lambda i=i: nc.sync.dma_start(
                    xt_sb[i][:], xt[i])]
            for i in range(6):
                side[(12 + 2 * i, -1)] = [lambda i=i: nc.sync.dma_start(
                    w2_sb[:, 4 * i:4 * (i + 1), :], w2[:, 4 * i:4 * (i + 1), :])]
            side_s = {(1, -1): [lambda: nc.sync.dma_start(w2s_sb[:], w2s[:])]}

            gemm1(MI, w13, xg_sb, LC, h_sb, side, chunk_outer_m0=True)
            gemm1(MS, w13s, xt_sb, LS, hs_sb, side_s)
            if DEBUG_DUMP:
                nc.sync.dma_start(h_dbg[:], h_sb[:])
            # Interleave the evict-heavy shared GEMM2 (many small psum
            # groups) with the evict-light routed GEMM2 (long psum
            # accumulations); end on an evict-light routed unit.
            r_units = list(gemm2T_units())
            s_units = list(gemm2_units())
            ns, nr = len(s_units), len(r_units)
            si = 0
            for ri, ru in enumerate(r_units):
                take = (ns * (ri + 1)) // nr
                while si < min(take, ns):
                    s_units[si]()
                    si += 1
                ru()
            while si < ns:
                s_units[si]()
                si += 1

    nc.compile()
    return nc


def _chunks_of(total, step):
    out, t0 = [], 0
    while t0 < total:
        ln = min(step, total - t0)
        out.append((t0, ln))
        t0 += ln
    return out


def _slabify(w):
    """[768, ncols] -> [ncols//128, 128, 6, 128] stationary slabs.

    slab[m, p, a, f] = w[a*128 + p, m*128 + f]
    """
    ncols = w.shape[1]
    return np.ascontiguousarray(
        w.reshape(DTILES, 128, ncols // 128, 128).transpose(2, 1, 0, 3))


def _pair_slabs(wg, wu):
    """Two [768, ncols] mats -> [ncols//128, 128, 2, 6, 128] slab pairs.

    Each pair is contiguous per partition (3072 B) so one dma_start
    streams a full w1/w3 slab pair.
    """
    m = wg.shape[1] // 128
    out = np.empty((m, 128, 2, DTILES, 128), dtype=BF16)
    out[:, :, 0] = _slabify(wg)
    out[:, :, 1] = _slabify(wu)
    return out


def _ptile(a):
    """[R, cols] with R = n*128 -> [128, n, cols] (partition-major)."""
    r, c = a.shape
    return np.ascontiguousarray(a.reshape(r // 128, 128, c).transpose(1, 0, 2))


def _xchunks(xrows, nch, L):
    """[ntok, D] f32 -> [nch, 128, 6, L] bf16 chunk-major tiles."""
    out = np.zeros((nch, 128, DTILES, L), dtype=BF16)
    for ci in range(nch):
        seg = xrows[ci * L:(ci + 1) * L]
        out[ci, :, :, :seg.shape[0]] = _ptile(seg.T.astype(BF16))
    return out


def kernel(**inputs) -> np.ndarray:
    global last_results
    x = np.asarray(inputs["x"], dtype=np.float32)
    gate_w = np.asarray(inputs["gate_w"], dtype=np.float32)
    gate_bias = np.asarray(inputs["gate_bias"], dtype=np.float32)
    w1 = np.asarray(inputs["w1"], dtype=np.float32)
    w2 = np.asarray(inputs["w2"], dtype=np.float32)
    w3 = np.asarray(inputs["w3"], dtype=np.float32)
    w1s = np.asarray(inputs["w1s"], dtype=np.float32)
    w2s = np.asarray(inputs["w2s"], dtype=np.float32)
    w3s = np.asarray(inputs["w3s"], dtype=np.float32)

    B, T, _ = x.shape
    N = B * T
    assert N == NTOK, f"kernel compiled for {NTOK} tokens, got {N}"
    flat = x.reshape(N, D)

    # ---- gate (host, f32, mirrors reference semantics) ----
    logits = flat @ gate_w                              # [N, E]
    scores = np.sqrt(np.logaddexp(np.float32(0.0), logits)).astype(np.float32)
    routed = scores + gate_bias
    idx = np.argsort(-routed, axis=1, kind="stable")[:, :TOPK]      # [N, K]
    wts = np.take_along_axis(scores, idx, axis=1)
    wts = wts / np.clip(wts.sum(axis=1, keepdims=True), 1e-6, None)

    # ---- dispatch: per-expert token lists ----
    ee = idx.reshape(-1)
    tok = np.repeat(np.arange(N), TOPK)
    ww = wts.reshape(-1).astype(np.float32)
    toks, cwts, counts = [], [], []
    for e in range(E):
        sel = ee == e
        toks.append(tok[sel])
        cwts.append(ww[sel])
        counts.append(int(sel.sum()))
    NC_, LC = _cap(max(counts))
    C = NC_ * LC

    # ---- per-core input maps ----
    xt_h = _xchunks(flat, NSC, LS)                      # [4, 128, 6, 512]
    in_maps = []
    for e in range(E):
        sl = slice(e * ISH, (e + 1) * ISH)
        in_maps.append({
            "w13": _pair_slabs(w1[e].astype(BF16), w3[e].astype(BF16)),
            "w2": _ptile(w2[e].astype(BF16)),           # [128, 24, 768]
            "w13s": _pair_slabs(w1s[:, sl].astype(BF16), w3s[:, sl].astype(BF16)),
            "w2s": _ptile(w2s[sl].astype(BF16)),        # [128, 3, 768]
            "xt": xt_h,
            "xg": _xchunks(flat[toks[e]], NC_, LC),     # [NC_, 128, 6, LC]
        })

    # ---- build + run ----
    key = (NC_, LC)
    if key not in _BUILD_CACHE:
        _BUILD_CACHE[key] = _build(NC_, LC)
    nc = _BUILD_CACHE[key]
    last_results = run_bass_kernel_spmd(nc, in_maps, core_ids=list(range(NCORES)))
    res = last_results.results

    # ---- combine (host): sum shared partials, scatter routed outputs ----
    # out_s: [128, 16, 768] partition-major bf16 partials -> [2048, 768] f32
    acc = res[0]["out_s"].astype(np.float32)
    for c in range(1, NCORES):
        acc += res[c]["out_s"].astype(np.float32)
    out = np.ascontiguousarray(acc.transpose(1, 0, 2)).reshape(N, D)
    for e in range(E):
        ce = counts[e]
        if ce:
            # out_r: [128, 6, C] bf16 -> [768, C] f32
            orr = res[e]["out_r"].astype(np.float32).transpose(1, 0, 2).reshape(D, C)
            out[toks[e]] += orr[:, :ce].T * cwts[e][:, None]
    return out.reshape(B, T, D).astype(np.float32)
